# revision 30
# baseline (speedup 1.0000x reference)
"""LIF neuron (STBP) forward kernel for Trainium2, 8-core data parallel.

Reference semantics (per element, scan over T):
    v = v * 0.9 + x_t
    s = (v >= 1.0)
    v = v - s * 1.0

Sharding: batch dim 32 -> 8 cores x 4; the recurrence is elementwise per
neuron so cores are independent.

Layout: per core the input is relayouted on host to partition-major
[P=128, T*2048] f32; the whole 16-MiB input lives in one SBUF arena.
Loads run per timestep (t0/t1 whole on the two HWDGE queues, half-column
pairs after) so data arrives in consumption order; the sync/scalar
engines carry ONLY loads so no compute-gated store ever head-of-line
blocks a load trigger. All stores go through SWDGE (gpsimd): spikes are
uint8 (4x less HBM write traffic than f32) and the host expands to f32.

Compute: the PRE-reset membrane u is the state, which folds the whole
step into one custom fused DVE op (registered below, per-NEFF uop table)
plus one stock tensor_scalar per step:

    LIF_U_ANT: u' = (u - (u >= th)) * beta + x   (reference rounding order)
    s8        = (u' >= th) -> uint8              (tensor_scalar, 2x mode)

For the last N_CAST timesteps even the tensor_scalar is dropped: the
state is shifted to w = u - 0.5 (host pre-biases x), stored directly via
SWDGE's f32->u8 cast (round-half-even, saturating), and the host decodes
spikes as (u8 >= 1), which equals (u >= 1) up to the measure-zero tie at
exactly u == 1.
"""

from contextlib import ExitStack

import numpy as np

import concourse.bacc as bacc
import concourse.mybir as mybir
import concourse.tile as tile
from concourse.bass_utils import run_bass_kernel_spmd

N_CORES = 8
B, T, C, H, W = 32, 16, 64, 32, 32
B_LOC = B // N_CORES  # 4 batches per core
P = 128               # SBUF partitions
F = (C * H * W) // P  # 512 free elements per partition per batch
FB = B_LOC * F        # 2048 free elements in a fused all-batch tile
BETA = 0.9
V_TH = 1.0

STORE_TS = [4, 4, 4, 3]     # SWDGE u8 store blocks covering t0..t14
N_CAST = 1                  # trailing timesteps stored via SWDGE f32->u8 cast

_CACHE = {}


def _get_lif_op():
    """Register (once) and return the fused LIF membrane-update DVE op."""
    import concourse.dve_ops as dve_ops
    from concourse.dve_ops import DveOp
    from concourse.dve_spec import C0, C1, Spec, Src0, Src1

    for o in dve_ops.OPS:
        if o.name == "LIF_U_ANT":
            return o

    op = DveOp(
        "LIF_U_ANT",
        Spec(
            body=(Src0 - (Src0 >= C1)) * C0 + Src1,
            reference=lambda in0, in1, s0, s1, imm2: (
                ((in0 - (in0 >= np.float32(s1)).astype(np.float32))
                 .astype(np.float32) * np.float32(s0) + in1).astype(np.float32)
            ),
        ),
        subdim=False,
        uops_sha={"v3": "5dffcaa405b6c09a", "v4": "7706b30f0e4fb094"},
    )
    dve_ops.OPS.append(op)
    dve_ops.CUSTOM_DVE_SPECS[op.name] = op.spec
    dve_ops._SUB_OPCODE_FOR_NAME[op.name] = (
        dve_ops._CUSTOM_DVE_ROW_BASE + len(dve_ops.OPS) - 1
    )
    return op


def _build(repeat: int = 1):
    lif_u = _get_lif_op()
    nc = bacc.Bacc(
        "TRN2", target_bir_lowering=False, debug=False, num_devices=N_CORES
    )
    x = nc.dram_tensor(
        "x", [P, T * FB], mybir.dt.float32, kind="ExternalInput"
    ).ap()
    s_out = nc.dram_tensor(
        "s", [P, T * FB], mybir.dt.uint8, kind="ExternalOutput"
    ).ap()

    with tile.TileContext(nc) as tc:
        _emit(nc, tc, x, s_out, repeat, lif_u)

    nc.compile()
    return nc


def _emit(nc, tc, x, s_out, repeat, lif_u):
    # HWDGE store block boundaries: after these timesteps
    store_ends = []
    acc = 0
    for nt in STORE_TS:
        acc += nt
        store_ends.append(acc)
    assert acc == T - N_CAST

    # SWDGE warm-up scratch (first gpsimd DMA pays Q7 setup; do it early,
    # off the critical path)
    warm = nc.dram_tensor("warm", [P, 4], mybir.dt.uint8).ap()

    with ExitStack() as ctx:
        xp = ctx.enter_context(tc.tile_pool(name="xp", bufs=1))
        sp = ctx.enter_context(tc.tile_pool(name="sp", bufs=1))
        up = ctx.enter_context(tc.tile_pool(name="up", bufs=3))
        wp = ctx.enter_context(tc.tile_pool(name="wp", bufs=1))

        h = FB // 2

        def xsl(t, half=None):
            a = t * FB
            if half is None:
                return slice(a, a + FB)
            return slice(a, a + h) if half == 0 else slice(a + h, a + FB)

        for _ in range(repeat):
            xall = xp.tile([P, T * FB], mybir.dt.float32)
            s8 = sp.tile([P, (T - N_CAST) * FB], mybir.dt.uint8)

            # t0 and t1 as WHOLE-timestep loads, one per HWDGE queue, so
            # both queues work on the first two steps concurrently and the
            # recurrence (which needs x0 AND x1) starts earliest. From t2
            # on, each timestep is half-column transfers on both queues so
            # data keeps arriving in consumption order. The sync/scalar
            # engines carry ONLY loads (stores go via SWDGE), so no store
            # wait ever head-of-line-blocks a load trigger.
            nc.sync.dma_start(xall[:, xsl(0)], x[:, xsl(0)])
            nc.scalar.dma_start(xall[:, xsl(1)], x[:, xsl(1)])
            for tp in range(2, T - 2, 2):
                nc.sync.dma_start(xall[:, xsl(tp)], x[:, xsl(tp)])
                nc.scalar.dma_start(xall[:, xsl(tp + 1)], x[:, xsl(tp + 1)])
            for t in (T - 2, T - 1):
                nc.sync.dma_start(xall[:, xsl(t, 0)], x[:, xsl(t, 0)])
                nc.scalar.dma_start(xall[:, xsl(t, 1)], x[:, xsl(t, 1)])

            wu = wp.tile([P, 4], mybir.dt.float32)
            nc.vector.memset(wu[:], 0.0)
            nc.gpsimd.dma_start(warm, wu[:])

            # For t >= T - N_CAST the state is w = u - 0.5 (host pre-biases
            # x there); the spike test in w-space is w >= 0.5, and the u8
            # round-half-even cast of w gives (u8 >= 1) == spike on host.
            def lif(t, out_ap, in0_ap, in1_ap):
                th = V_TH if t <= T - N_CAST else V_TH - 0.5
                nc.vector._custom_dve(
                    lif_u, out=out_ap, in0=in0_ap, in1=in1_ap,
                    s0=BETA, s1=th,
                )

            def isge(t, out_ap, in_ap):
                nc.vector.tensor_scalar(
                    out_ap, in_ap, V_TH, None, mybir.AluOpType.is_ge
                )

            u = xall[:, xsl(0)]
            isge(0, s8[:, xsl(0)], u)
            prev = 0
            for t in range(T):
                if t > 0:
                    if t < T - 1:
                        un = up.tile([P, FB], mybir.dt.float32)
                        lif(t, un[:], u, xall[:, xsl(t)])
                        u = un[:]
                    else:
                        # final step split in quarters so LIF -> cast-store
                        # slices pipeline right behind the last x arrivals
                        un = up.tile([P, FB], mybir.dt.float32)
                        q = FB // 4
                        for k in range(4):
                            sl = slice(k * q, (k + 1) * q)
                            gsl = slice(t * FB + k * q, t * FB + (k + 1) * q)
                            lif(t, un[:, sl], u[:, sl], xall[:, gsl])
                            nc.gpsimd.dma_start(s_out[:, gsl], un[:, sl])
                        continue
                    if t < T - N_CAST:
                        isge(t, s8[:, xsl(t)], u)
                    else:
                        # w-space: store the membrane itself, cast f32->u8
                        nc.gpsimd.dma_start(s_out[:, xsl(t)], u)
                if t < T - N_CAST and t + 1 in store_ends:
                    sl = slice(prev * FB, (t + 1) * FB)
                    nc.gpsimd.dma_start(s_out[:, sl], s8[:, sl])
                    prev = t + 1


def _get_nc(repeat: int = 1):
    key = f"nc{repeat}"
    if key not in _CACHE:
        _CACHE[key] = _build(repeat)
    return _CACHE[key]


def _shard_input(x_seq: np.ndarray, i: int) -> np.ndarray:
    # [4, T, C, H, W] -> partition-major arena layout [P, T*B_LOC*F].
    # The trailing N_CAST timesteps run in w = u - 0.5 space (so the u8
    # cast-store encodes the spike): entry step gets x - 0.5, later steps
    # x - 0.5*(1 - beta).
    xc = x_seq[i * B_LOC:(i + 1) * B_LOC].reshape(B_LOC, T, P, F)
    out = np.ascontiguousarray(
        xc.transpose(2, 1, 0, 3).reshape(P, T, FB)
    )
    out[:, T - N_CAST] -= np.float32(0.5)
    for t in range(T - N_CAST + 1, T):
        out[:, t] -= np.float32(0.5 * (1.0 - BETA))
    return out.reshape(P, T * FB)


def _unshard_output(s_u8: np.ndarray) -> np.ndarray:
    # [P, T*B_LOC*F] u8 -> [B_LOC, T, C, H, W] f32 spikes.
    # For t < T-N_CAST the byte is the is_ge result (0/1); for the cast
    # tail it is round-half-even(w) which is >= 1 exactly when w > 0.5
    # (i.e. u > 1.0 up to the half-even tie at exactly 1.0).
    s = (s_u8.reshape(P, T, B_LOC, F) >= 1).astype(np.float32)
    return s.transpose(2, 1, 0, 3).reshape(B_LOC, T, C, H, W)


def _run(x_seq: np.ndarray, trace: bool = False, repeat: int = 1):
    """Shard, execute on 8 cores, gather. Returns (output, BassKernelResults)."""
    nc = _get_nc(repeat)
    x_seq = np.ascontiguousarray(x_seq, dtype=np.float32)
    in_maps = [{"x": _shard_input(x_seq, i)} for i in range(N_CORES)]
    res = run_bass_kernel_spmd(
        nc, in_maps, core_ids=list(range(N_CORES)), trace=trace
    )
    out = np.concatenate(
        [_unshard_output(r["s"]) for r in res.results], axis=0
    )
    return out, res


def kernel(x_seq: np.ndarray) -> np.ndarray:
    out, _ = _run(x_seq, trace=False)
    return out


# revision 31
# speedup vs baseline: 1.0688x; 1.0688x over previous
"""LIF neuron (STBP) forward kernel for Trainium2, 8-core data parallel.

Reference semantics (per element, scan over T):
    v = v * 0.9 + x_t
    s = (v >= 1.0)
    v = v - s * 1.0

Sharding: batch dim 32 -> 8 cores x 4; the recurrence is elementwise per
neuron so cores are independent.

Layout: per core the input is relayouted on host to partition-major
[P=128, T*2048] f32; the whole 16-MiB input lives in one SBUF arena.
Loads run per timestep (t0/t1 whole on the two HWDGE queues, half-column
pairs after) so data arrives in consumption order; the sync/scalar
engines carry ONLY loads so no compute-gated store ever head-of-line
blocks a load trigger. All stores go through SWDGE (gpsimd): spikes are
uint8 (4x less HBM write traffic than f32) and the host expands to f32.

Compute: the PRE-reset membrane u is the state, which folds the whole
step into one custom fused DVE op (registered below, per-NEFF uop table)
plus one stock tensor_scalar per step:

    LIF_U_ANT: u' = (u - (u >= th)) * beta + x   (reference rounding order)
    s8        = (u' >= th) -> uint8              (tensor_scalar, 2x mode)

For the last N_CAST timesteps even the tensor_scalar is dropped: the
state is shifted to w = u - 0.5 (host pre-biases x), stored directly via
SWDGE's f32->u8 cast (round-half-even, saturating), and the host decodes
spikes as (u8 >= 1), which equals (u >= 1) up to the measure-zero tie at
exactly u == 1.
"""

from contextlib import ExitStack

import numpy as np

import concourse.bacc as bacc
import concourse.mybir as mybir
import concourse.tile as tile
from concourse.bass_utils import run_bass_kernel_spmd

N_CORES = 8
B, T, C, H, W = 32, 16, 64, 32, 32
B_LOC = B // N_CORES  # 4 batches per core
P = 128               # SBUF partitions
F = (C * H * W) // P  # 512 free elements per partition per batch
FB = B_LOC * F        # 2048 free elements in a fused all-batch tile
BETA = 0.9
V_TH = 1.0

STORE_TS = [4, 4, 4]        # SWDGE u8 store blocks covering t0..t11
N_CAST = 4                  # trailing timesteps stored via SWDGE f32->u8 cast

_CACHE = {}


def _get_lif_op():
    """Register (once) and return the fused LIF membrane-update DVE op."""
    import concourse.dve_ops as dve_ops
    from concourse.dve_ops import DveOp
    from concourse.dve_spec import C0, C1, Spec, Src0, Src1

    for o in dve_ops.OPS:
        if o.name == "LIF_U_ANT":
            return o

    op = DveOp(
        "LIF_U_ANT",
        Spec(
            body=(Src0 - (Src0 >= C1)) * C0 + Src1,
            reference=lambda in0, in1, s0, s1, imm2: (
                ((in0 - (in0 >= np.float32(s1)).astype(np.float32))
                 .astype(np.float32) * np.float32(s0) + in1).astype(np.float32)
            ),
        ),
        subdim=False,
        uops_sha={"v3": "5dffcaa405b6c09a", "v4": "7706b30f0e4fb094"},
    )
    dve_ops.OPS.append(op)
    dve_ops.CUSTOM_DVE_SPECS[op.name] = op.spec
    dve_ops._SUB_OPCODE_FOR_NAME[op.name] = (
        dve_ops._CUSTOM_DVE_ROW_BASE + len(dve_ops.OPS) - 1
    )
    return op


def _build(repeat: int = 1):
    lif_u = _get_lif_op()
    nc = bacc.Bacc(
        "TRN2", target_bir_lowering=False, debug=False, num_devices=N_CORES
    )
    x = nc.dram_tensor(
        "x", [P, T * FB], mybir.dt.float32, kind="ExternalInput"
    ).ap()
    s_out = nc.dram_tensor(
        "s", [P, T * FB], mybir.dt.uint8, kind="ExternalOutput"
    ).ap()

    with tile.TileContext(nc) as tc:
        _emit(nc, tc, x, s_out, repeat, lif_u)

    nc.compile()
    return nc


def _emit(nc, tc, x, s_out, repeat, lif_u):
    # HWDGE store block boundaries: after these timesteps
    store_ends = []
    acc = 0
    for nt in STORE_TS:
        acc += nt
        store_ends.append(acc)
    assert acc == T - N_CAST

    # SWDGE warm-up scratch (first gpsimd DMA pays Q7 setup; do it early,
    # off the critical path)
    warm = nc.dram_tensor("warm", [P, 4], mybir.dt.uint8).ap()

    with ExitStack() as ctx:
        xp = ctx.enter_context(tc.tile_pool(name="xp", bufs=1))
        sp = ctx.enter_context(tc.tile_pool(name="sp", bufs=1))
        up = ctx.enter_context(tc.tile_pool(name="up", bufs=3))
        wp = ctx.enter_context(tc.tile_pool(name="wp", bufs=1))

        h = FB // 2

        def xsl(t, half=None):
            a = t * FB
            if half is None:
                return slice(a, a + FB)
            return slice(a, a + h) if half == 0 else slice(a + h, a + FB)

        for _ in range(repeat):
            xall = xp.tile([P, T * FB], mybir.dt.float32)
            s8 = sp.tile([P, (T - N_CAST) * FB], mybir.dt.uint8)

            # t0 and t1 as WHOLE-timestep loads, one per HWDGE queue, so
            # both queues work on the first two steps concurrently and the
            # recurrence (which needs x0 AND x1) starts earliest. From t2
            # on, each timestep is half-column transfers on both queues so
            # data keeps arriving in consumption order. The sync/scalar
            # engines carry ONLY loads (stores go via SWDGE), so no store
            # wait ever head-of-line-blocks a load trigger.
            nc.sync.dma_start(xall[:, xsl(0)], x[:, xsl(0)])
            nc.scalar.dma_start(xall[:, xsl(1)], x[:, xsl(1)])
            for tp in range(2, T - 2, 2):
                nc.sync.dma_start(xall[:, xsl(tp)], x[:, xsl(tp)])
                nc.scalar.dma_start(xall[:, xsl(tp + 1)], x[:, xsl(tp + 1)])
            for t in (T - 2, T - 1):
                nc.sync.dma_start(xall[:, xsl(t, 0)], x[:, xsl(t, 0)])
                nc.scalar.dma_start(xall[:, xsl(t, 1)], x[:, xsl(t, 1)])

            wu = wp.tile([P, 4], mybir.dt.float32)
            nc.vector.memset(wu[:], 0.0)
            nc.gpsimd.dma_start(warm, wu[:])

            # For t >= T - N_CAST the state is w = u - 0.5 (host pre-biases
            # x there); the spike test in w-space is w >= 0.5, and the u8
            # round-half-even cast of w gives (u8 >= 1) == spike on host.
            def lif(t, out_ap, in0_ap, in1_ap):
                th = V_TH if t <= T - N_CAST else V_TH - 0.5
                nc.vector._custom_dve(
                    lif_u, out=out_ap, in0=in0_ap, in1=in1_ap,
                    s0=BETA, s1=th,
                )

            def isge(t, out_ap, in_ap):
                nc.vector.tensor_scalar(
                    out_ap, in_ap, V_TH, None, mybir.AluOpType.is_ge
                )

            u = xall[:, xsl(0)]
            isge(0, s8[:, xsl(0)], u)
            prev = 0
            for t in range(T):
                if t > 0:
                    if t < T - 1:
                        un = up.tile([P, FB], mybir.dt.float32)
                        lif(t, un[:], u, xall[:, xsl(t)])
                        u = un[:]
                    else:
                        # final step split in quarters so LIF -> cast-store
                        # slices pipeline right behind the last x arrivals
                        un = up.tile([P, FB], mybir.dt.float32)
                        q = FB // 4
                        for k in range(4):
                            sl = slice(k * q, (k + 1) * q)
                            gsl = slice(t * FB + k * q, t * FB + (k + 1) * q)
                            lif(t, un[:, sl], u[:, sl], xall[:, gsl])
                            nc.gpsimd.dma_start(s_out[:, gsl], un[:, sl])
                        continue
                    if t < T - N_CAST:
                        isge(t, s8[:, xsl(t)], u)
                    else:
                        # w-space: store the membrane itself, cast f32->u8
                        nc.gpsimd.dma_start(s_out[:, xsl(t)], u)
                if t < T - N_CAST and t + 1 in store_ends:
                    sl = slice(prev * FB, (t + 1) * FB)
                    nc.gpsimd.dma_start(s_out[:, sl], s8[:, sl])
                    prev = t + 1


def _get_nc(repeat: int = 1):
    key = f"nc{repeat}"
    if key not in _CACHE:
        _CACHE[key] = _build(repeat)
    return _CACHE[key]


def _shard_input(x_seq: np.ndarray, i: int) -> np.ndarray:
    # [4, T, C, H, W] -> partition-major arena layout [P, T*B_LOC*F].
    # The trailing N_CAST timesteps run in w = u - 0.5 space (so the u8
    # cast-store encodes the spike): entry step gets x - 0.5, later steps
    # x - 0.5*(1 - beta).
    xc = x_seq[i * B_LOC:(i + 1) * B_LOC].reshape(B_LOC, T, P, F)
    out = np.ascontiguousarray(
        xc.transpose(2, 1, 0, 3).reshape(P, T, FB)
    )
    out[:, T - N_CAST] -= np.float32(0.5)
    for t in range(T - N_CAST + 1, T):
        out[:, t] -= np.float32(0.5 * (1.0 - BETA))
    return out.reshape(P, T * FB)


def _unshard_output(s_u8: np.ndarray) -> np.ndarray:
    # [P, T*B_LOC*F] u8 -> [B_LOC, T, C, H, W] f32 spikes.
    # For t < T-N_CAST the byte is the is_ge result (0/1); for the cast
    # tail it is round-half-even(w) which is >= 1 exactly when w > 0.5
    # (i.e. u > 1.0 up to the half-even tie at exactly 1.0).
    s = (s_u8.reshape(P, T, B_LOC, F) >= 1).astype(np.float32)
    return s.transpose(2, 1, 0, 3).reshape(B_LOC, T, C, H, W)


def _run(x_seq: np.ndarray, trace: bool = False, repeat: int = 1):
    """Shard, execute on 8 cores, gather. Returns (output, BassKernelResults)."""
    nc = _get_nc(repeat)
    x_seq = np.ascontiguousarray(x_seq, dtype=np.float32)
    in_maps = [{"x": _shard_input(x_seq, i)} for i in range(N_CORES)]
    res = run_bass_kernel_spmd(
        nc, in_maps, core_ids=list(range(N_CORES)), trace=trace
    )
    out = np.concatenate(
        [_unshard_output(r["s"]) for r in res.results], axis=0
    )
    return out, res


def kernel(x_seq: np.ndarray) -> np.ndarray:
    out, _ = _run(x_seq, trace=False)
    return out
